# revision 8
# baseline (speedup 1.0000x reference)
"""Causal self-attention (B=2,T=2048,D=1024,H=16,HD=64) + RoPE on 8 TRN2 NeuronCores.

Sharding: core = b*4 + g  (b: batch, g: head-group of 4 heads).
Each core computes QKV projection for its 4 heads, causal attention, and a
partial out-projection (rank-256 contribution). Host sums the 4 partials per
batch (the "all-reduce after out_proj").

v2: all matmul operands bf16 (same PE streaming rate as fp32r, but FWL weight
loads, half DMA bytes, 2x DVE modes). RoPE partner-swap moved off DVE onto
SBUF->SBUF DMA; causal masks on gpsimd; exp on ACT (the only exp engine).
Attention j-steps are software-pipelined (scores j+1 issued before PV j) and
interleaved with QKV/out-proj matmul "filler" so the PE never idles long
enough for HAM to re-throttle to half clock.
"""
import numpy as np
import ml_dtypes

import concourse.bass as bass
import concourse.mybir as mybir
from concourse import bacc
from concourse.tile import TileContext
from concourse.bass_utils import run_bass_kernel_spmd

B, T, D, H = 2, 2048, 1024, 16
HD = D // H            # 64
G = 4                  # head groups (tensor-parallel factor)
HPG = H // G           # 4 heads per group
DG = HPG * HD          # 256 head-dims per group
KC = D // 128          # 8 contraction chunks for D
NT = T // 512          # 4 T-chunks of 512
TT = T // 128          # 16 T-tiles of 128
F32 = mybir.dt.float32
BF = mybir.dt.bfloat16
NPBF = ml_dtypes.bfloat16

_CACHE = {}
DEBUG_DUMPS = False


def _build():
    nc = bacc.Bacc("TRN2", target_bir_lowering=False, debug=False, num_devices=8)

    xT_d = nc.dram_tensor("xT", [128, KC, T], BF, kind="ExternalInput").ap()
    wqk_d = nc.dram_tensor("wqk", [128, KC, 2 * DG], BF, kind="ExternalInput").ap()
    wv_d = nc.dram_tensor("wv", [128, KC, DG], BF, kind="ExternalInput").ap()
    wout_d = nc.dram_tensor("wout", [128, 2, D], BF, kind="ExternalInput").ap()
    cos_d = nc.dram_tensor("cos128", [128, T], BF, kind="ExternalInput").ap()
    sin_d = nc.dram_tensor("sin128s", [128, T], BF, kind="ExternalInput").ap()
    tri_d = nc.dram_tensor("tri2", [128, 256], BF, kind="ExternalInput").ap()
    out_d = nc.dram_tensor("out", [T, D], BF, kind="ExternalOutput").ap()

    with TileContext(nc) as tc:
        with (
            tc.tile_pool(name="const", bufs=1) as cpool,
            tc.tile_pool(name="big", bufs=1) as big,
            tc.tile_pool(name="rope", bufs=2) as ropep,
            tc.tile_pool(name="work", bufs=2) as work,
            tc.tile_pool(name="expp", bufs=3) as expp,
            tc.tile_pool(name="outp", bufs=2) as outp,
            tc.tile_pool(name="ps_mm", bufs=2, space="PSUM") as ps_mm,
            tc.tile_pool(name="ps_sc", bufs=2, space="PSUM") as ps_sc,
            tc.tile_pool(name="ps_pv", bufs=1, space="PSUM") as ps_pv,
        ):
            cos_sb = cpool.tile([128, T], BF)
            sin_sb = cpool.tile([128, T], BF)
            tri_sb = cpool.tile([128, 256], BF)
            xT_sb = big.tile([128, KC, T], BF)
            wqk_sb = big.tile([128, KC, 2 * DG], BF)
            wv_sb = big.tile([128, KC, DG], BF)
            wout_sb = big.tile([128, 2, D], BF)
            # first q/k matmuls need wqk + xT chunk 0; order DMAs accordingly
            nc.sync.dma_start(out=wqk_sb[:], in_=wqk_d)
            nc.sync.dma_start(out=xT_sb[:, :, 0:512], in_=xT_d[:, :, 0:512])
            nc.sync.dma_start(out=wv_sb[:], in_=wv_d)
            nc.sync.dma_start(out=cos_sb[:], in_=cos_d)
            nc.sync.dma_start(out=sin_sb[:], in_=sin_d)
            nc.sync.dma_start(
                out=xT_sb[:, :, 512:1024], in_=xT_d[:, :, 512:1024])
            nc.sync.dma_start(out=tri_sb[:], in_=tri_d)
            nc.sync.dma_start(
                out=xT_sb[:, :, 1024:1536], in_=xT_d[:, :, 1024:1536])
            nc.sync.dma_start(
                out=xT_sb[:, :, 1536:2048], in_=xT_d[:, :, 1536:2048])
            nc.sync.dma_start(out=wout_sb[:], in_=wout_d)

            # PE warm-up: dummy matmuls fill the DMA lead-in so HAM unthrottles
            warm_sb = cpool.tile([128, 256], BF)
            nc.vector.memset(warm_sb[:].bitcast(F32), 0.0)
            for w in range(28):
                wp = ps_mm.tile([128, 512], F32, tag="mm")
                nc.tensor.matmul(
                    wp[:, 0:256], lhsT=warm_sb[:, 0:128], rhs=warm_sb[:],
                    start=True, stop=True,
                )

            # qkT_sb m-index: 0,1 = q head-pairs (0,1),(2,3); 2,3 = k pairs
            qkT_sb = big.tile([128, 4, T], BF)
            # HD+2 so the ones columns pair up into one f32 lane for memset:
            # 0x3F803F80 == two bf16 1.0s
            v_sb = big.tile([128, TT, HPG, HD + 2], BF)
            ones_f32 = float(np.frombuffer(
                np.uint32(0x3F803F80).tobytes(), dtype=np.float32)[0])
            nc.vector.memset(
                v_sb[:, :, :, HD:HD + 2].bitcast(F32), ones_f32)
            outT_sb = big.tile([128, 2, T], BF)

            def qk_unit(n, m, qkr):
                ns = slice(n * 512, (n + 1) * 512)
                ps = ps_mm.tile([128, 512], F32, tag="mm")
                for k in range(KC):
                    nc.tensor.matmul(
                        ps[:],
                        lhsT=wqk_sb[:, k, m * 128:(m + 1) * 128],
                        rhs=xT_sb[:, k, ns],
                        start=(k == 0),
                        stop=(k == KC - 1),
                    )
                nc.vector.tensor_copy(qkr[:, m, :], ps[:])

            def rope_finish(n, qkr):
                # partner swap p <-> p^16 via SBUF->SBUF DMA (frees DVE)
                ns = slice(n * 512, (n + 1) * 512)
                swp = ropep.tile([128, 4, 512], BF, tag="swp")
                for q in range(4):
                    nc.sync.dma_start(
                        out=swp[32 * q + 16:32 * q + 32], in_=qkr[32 * q:32 * q + 16])
                    nc.sync.dma_start(
                        out=swp[32 * q:32 * q + 16], in_=qkr[32 * q + 16:32 * q + 32])
                for m in range(4):
                    nc.vector.tensor_mul(qkr[:, m, :], qkr[:, m, :], cos_sb[:, ns])
                    nc.vector.tensor_mul(swp[:, m, :], swp[:, m, :], sin_sb[:, ns])
                    nc.vector.tensor_add(
                        qkT_sb[:, m, ns], qkr[:, m, :], swp[:, m, :])

            def v_unit(n, j):
                ps = ps_mm.tile([128, 256], F32, tag="mm")
                for k in range(KC):
                    nc.tensor.matmul(
                        ps[:],
                        lhsT=xT_sb[:, k, j * 128:(j + 1) * 128],
                        rhs=wv_sb[:, k, :],
                        start=(k == 0),
                        stop=(k == KC - 1),
                    )
                nc.vector.tensor_copy(
                    v_sb[:, j, :, 0:HD], ps[:].rearrange("p (h d) -> p h d", h=HPG)
                )

            def qkv_units(n):
                """Filler units (closures) for projecting chunk n."""
                qkr = [None]

                def qk_closure(m):
                    def f():
                        if m == 0:
                            qkr_t = ropep.tile([128, 4, 512], BF, tag="qkr")
                            qkr[0] = qkr_t
                        qk_unit(n, m, qkr[0])
                        if m == 3:
                            rope_finish(n, qkr[0])
                    return f
                return [qk_closure(m) for m in range(4)] + \
                       [lambda j=j: v_unit(n, j) for j in range(4 * n, 4 * n + 4)]

            def proj_unit(t, nh):
                ps = ps_mm.tile([128, 512], F32, tag="mm")
                for c in range(2):
                    nc.tensor.matmul(
                        ps[:],
                        lhsT=outT_sb[:, c, t * 128:(t + 1) * 128],
                        rhs=wout_sb[:, c, nh * 512:(nh + 1) * 512],
                        start=(c == 0),
                        stop=(c == 1),
                    )
                ot = outp.tile([128, 512], BF, tag="ot")
                if (t + nh) % 2 == 0:
                    nc.scalar.copy(out=ot[:], in_=ps[:])
                else:
                    nc.vector.tensor_copy(ot[:], ps[:])
                nc.sync.dma_start(
                    out=out_d[t * 128:(t + 1) * 128, nh * 512:(nh + 1) * 512],
                    in_=ot[:],
                )

            def proj_units(g):
                return [lambda t=t, nh=nh: proj_unit(t, nh)
                        for t in range(4 * g, 4 * g + 4) for nh in range(2)]

            fill_q = []

            def pump(k=1):
                for _ in range(k):
                    if fill_q:
                        fill_q.pop(0)()

            def attn_group(g, pat):
                """pat(step) -> how many filler units to pump after that step."""
                step = [0]
                for hp in range(2):
                    qm, km = hp, 2 + hp
                    pv0 = ps_pv.tile([65, 512], F32, tag="pv0")
                    pv1 = ps_pv.tile([65, 512], F32, tag="pv1")
                    jmax = 4 * g + 3
                    exs = {}

                    def pv_step(j):
                        ex = exs.pop(j)
                        d = j - 4 * g
                        nstart = 128 * d if d > 0 else 0
                        ncols = 512 - nstart
                        for half, pv in ((0, pv0), (1, pv1)):
                            nc.tensor.matmul(
                                pv[:, nstart:512],
                                lhsT=v_sb[:, j, 2 * hp + half, 0:HD + 1],
                                rhs=ex[:, half * 512:half * 512 + ncols],
                                start=(j == 0),
                                stop=(j == jmax),
                            )

                    for j in range(jmax + 1):
                        d = j - 4 * g
                        nstart = 128 * d if d > 0 else 0
                        ncols = 512 - nstart
                        ex = expp.tile([128, 1024], BF, tag="ex")
                        exs[j] = ex
                        # two heads' score matmuls packed into one PE pass
                        # (row groups 0-1 / 2-3 run concurrently)
                        sc = ps_sc.tile([128, 1024], F32, tag="sc")
                        for half in range(2):
                            pb = 64 * half
                            nc.tensor.matmul(
                                sc[:, half * 512:half * 512 + ncols],
                                lhsT=qkT_sb[pb:pb + 64, km, j * 128:(j + 1) * 128],
                                rhs=qkT_sb[pb:pb + 64, qm,
                                           g * 512 + nstart:(g + 1) * 512],
                                start=True,
                                stop=True,
                            )
                        if ncols == 512:
                            nc.scalar.activation(
                                ex[:], sc[:],
                                mybir.ActivationFunctionType.Exp, scale=0.125,
                            )
                        else:
                            exv = ex[:].rearrange("p (u c) -> p u c", u=2)[:, :, 0:ncols]
                            scv = sc[:].rearrange("p (u c) -> p u c", u=2)[:, :, 0:ncols]
                            nc.scalar.activation(
                                exv, scv, mybir.ActivationFunctionType.Exp, scale=0.125,
                            )
                        if d >= 0:
                            # causal mask on the diagonal 128x128 of both heads
                            exm = ex[:].rearrange("p (u c) -> p u c", u=2)[:, :, 0:128]
                            trm = tri_sb[:].rearrange("p (u c) -> p u c", u=2)
                            nc.gpsimd.tensor_mul(exm, exm, trm)
                        if j > 0:
                            pv_step(j - 1)
                        pump(pat(step[0]))
                        step[0] += 1
                    pv_step(jmax)

                    for half, pv in ((0, pv0), (1, pv1)):
                        pb = 64 * half
                        den = work.tile([1, 512], F32, tag="den")
                        nc.vector.tensor_copy(den[:], pv[64:65, :])
                        rec = work.tile([1, 512], F32, tag="rec")
                        nc.vector.reciprocal_approx_fast(rec[:], den[:])
                        recb = work.tile([64, 512], F32, tag="recb")
                        nc.gpsimd.partition_broadcast(recb[:], rec[0:1, :], channels=64)
                        nc.vector.tensor_mul(
                            outT_sb[pb:pb + 64, hp, g * 512:(g + 1) * 512],
                            pv[0:64, :],
                            recb[:],
                        )

            # lead-in: project chunk 0 densely, then attention with filler
            for u in qkv_units(0):
                u()
            fill_q.extend(qkv_units(1))
            attn_group(0, lambda s: 1)            # 8 steps, 8 units
            fill_q.extend(qkv_units(2))
            attn_group(1, lambda s: s % 2)        # 16 steps, 8 units
            fill_q.extend(qkv_units(3))
            fill_q.extend(proj_units(0))
            attn_group(2, lambda s: 1 if s % 3 != 2 else 0)  # 24 steps, 16 units
            fill_q.extend(proj_units(1))
            fill_q.extend(proj_units(2))
            attn_group(3, lambda s: (s + 1) % 2)  # 32 steps, 16 units
            pump(len(fill_q))
            for u in proj_units(3):
                u()

            if DEBUG_DUMPS:
                dq = nc.dram_tensor("dbg_qk", [128, 4, T], BF, kind="ExternalOutput").ap()
                dv = nc.dram_tensor("dbg_v", [128, TT, HPG, HD + 2], BF, kind="ExternalOutput").ap()
                do = nc.dram_tensor("dbg_outT", [128, 2, T], BF, kind="ExternalOutput").ap()
                nc.sync.dma_start(out=dq[:], in_=qkT_sb[:])
                nc.sync.dma_start(out=dv[:], in_=v_sb[:])
                nc.sync.dma_start(out=do[:], in_=outT_sb[:])

    nc.compile()
    return nc


def _qk_perm():
    """hd permutation for q/k columns: RoPE pair j -> (e,o) rows 16-interleaved
    so the swap stays within 32-partition quadrants (p <-> p^16)."""
    perm = np.empty(HD, dtype=np.int64)
    for p in range(HD):
        q32, i = divmod(p, 32)
        j = 16 * q32 + (i % 16)
        perm[p] = 2 * j + (1 if i >= 16 else 0)
    return perm


def _prepare_shards(x, w_qkv, w_out, freqs_cos, freqs_sin):
    perm = _qk_perm()
    cosT = np.ascontiguousarray(freqs_cos.T)  # [32, T]
    sinT = np.ascontiguousarray(freqs_sin.T)
    # row p of a 64-row head block: pair j = 16*(p//32 % 2) + p%16, sign -/+ for e/o
    cos128 = np.empty((128, T), dtype=np.float32)
    sin128s = np.empty((128, T), dtype=np.float32)
    for p in range(128):
        ph = p % 64
        q32, i = divmod(ph, 32)
        j = 16 * q32 + (i % 16)
        cos128[p] = cosT[j]
        sin128s[p] = sinT[j] * (-1.0 if i < 16 else 1.0)
    kk, qq = np.meshgrid(np.arange(128), np.arange(128), indexing="ij")
    tri = (kk <= qq).astype(np.float32)
    tri2 = np.concatenate([tri, tri], axis=1)  # [128, 256] for both heads

    w3 = w_qkv.reshape(D, 3, H, HD)
    in_maps = []
    for core in range(8):
        b, g = divmod(core, G)
        heads = np.arange(g * HPG, (g + 1) * HPG)
        wq = w3[:, 0, heads][:, :, perm].reshape(D, DG)
        wk = w3[:, 1, heads][:, :, perm].reshape(D, DG)
        wqk = np.ascontiguousarray(np.concatenate([wq, wk], axis=1))
        wv = np.ascontiguousarray(w3[:, 2, heads].reshape(D, DG))
        wo = np.ascontiguousarray(w_out.reshape(H, HD, D)[heads].reshape(DG, D))
        def sb_layout(a, kc=KC):
            # [128*kc, F] -> [128, kc, F] with partition-major contiguity
            return np.ascontiguousarray(
                a.reshape(kc, 128, -1).transpose(1, 0, 2)).astype(NPBF)
        in_maps.append({
            "xT": sb_layout(x[b].T),
            "wqk": sb_layout(wqk),
            "wv": sb_layout(wv),
            "wout": sb_layout(wo, kc=2),
            "cos128": cos128.astype(NPBF),
            "sin128s": sin128s.astype(NPBF),
            "tri2": tri2.astype(NPBF),
        })
    return in_maps


def _run(in_maps, **kw):
    if "nc" not in _CACHE:
        _CACHE["nc"] = _build()
    return run_bass_kernel_spmd(_CACHE["nc"], in_maps, core_ids=list(range(8)), **kw)


def kernel(x, w_qkv, w_out, freqs_cos, freqs_sin):
    x = np.asarray(x, dtype=np.float32)
    w_qkv = np.asarray(w_qkv, dtype=np.float32)
    w_out = np.asarray(w_out, dtype=np.float32)
    freqs_cos = np.asarray(freqs_cos, dtype=np.float32)
    freqs_sin = np.asarray(freqs_sin, dtype=np.float32)

    in_maps = _prepare_shards(x, w_qkv, w_out, freqs_cos, freqs_sin)
    res = _run(in_maps)
    out = np.zeros((B, T, D), dtype=np.float64)
    for core in range(8):
        out[core // G] += res.results[core]["out"].astype(np.float64)
    return out.astype(np.float32)


# revision 11
# speedup vs baseline: 1.2760x; 1.2760x over previous
"""Causal self-attention (B=2,T=2048,D=1024,H=16,HD=64) + RoPE on 8 TRN2 NeuronCores.

Sharding: core = b*4 + g  (b: batch, g: head-group of 4 heads).
Each core computes QKV projection for its 4 heads, causal attention, and a
partial out-projection (rank-256 contribution). Host sums the 4 partials per
batch (the "all-reduce after out_proj").

v2: all matmul operands bf16 (same PE streaming rate as fp32r, but FWL weight
loads, half DMA bytes, 2x DVE modes). RoPE partner-swap moved off DVE onto
SBUF->SBUF DMA; causal masks on gpsimd; exp on ACT (the only exp engine).
Attention j-steps are software-pipelined (scores j+1 issued before PV j) and
interleaved with QKV/out-proj matmul "filler" so the PE never idles long
enough for HAM to re-throttle to half clock.
"""
import numpy as np
import ml_dtypes

import concourse.bass as bass
import concourse.mybir as mybir
from concourse import bacc
from concourse.tile import TileContext
from concourse.bass_utils import run_bass_kernel_spmd

B, T, D, H = 2, 2048, 1024, 16
HD = D // H            # 64
G = 4                  # head groups (tensor-parallel factor)
HPG = H // G           # 4 heads per group
DG = HPG * HD          # 256 head-dims per group
KC = D // 128          # 8 contraction chunks for D
NT = T // 512          # 4 T-chunks of 512
TT = T // 128          # 16 T-tiles of 128
F32 = mybir.dt.float32
BF = mybir.dt.bfloat16
NPBF = ml_dtypes.bfloat16

_CACHE = {}
DEBUG_DUMPS = False


def _build():
    nc = bacc.Bacc("TRN2", target_bir_lowering=False, debug=False, num_devices=8)

    xT_d = nc.dram_tensor("xT", [128, KC, T], BF, kind="ExternalInput").ap()
    wqk_d = nc.dram_tensor("wqk", [128, KC, 2 * DG], BF, kind="ExternalInput").ap()
    wv_d = nc.dram_tensor("wv", [128, KC, DG], BF, kind="ExternalInput").ap()
    wout_d = nc.dram_tensor("wout", [128, 2, D], BF, kind="ExternalInput").ap()
    cos_d = nc.dram_tensor("cos128", [128, T], BF, kind="ExternalInput").ap()
    sin_d = nc.dram_tensor("sin128s", [128, T], BF, kind="ExternalInput").ap()
    tri_d = nc.dram_tensor("tri2", [128, 256], BF, kind="ExternalInput").ap()
    out_d = nc.dram_tensor("out", [T, D], BF, kind="ExternalOutput").ap()

    with TileContext(nc) as tc:
        with (
            tc.tile_pool(name="const", bufs=1) as cpool,
            tc.tile_pool(name="big", bufs=1) as big,
            tc.tile_pool(name="rope", bufs=2) as ropep,
            tc.tile_pool(name="work", bufs=2) as work,
            tc.tile_pool(name="expp", bufs=3) as expp,
            tc.tile_pool(name="outp", bufs=2) as outp,
            tc.tile_pool(name="ps_mm", bufs=1, space="PSUM") as ps_mm,
            tc.tile_pool(name="ps_sc", bufs=2, space="PSUM") as ps_sc,
            tc.tile_pool(name="ps_pv0", bufs=2, space="PSUM") as ps_pv0,
            tc.tile_pool(name="ps_pv1", bufs=1, space="PSUM") as ps_pv1,
        ):
            cos_sb = cpool.tile([128, T], BF)
            sin_sb = cpool.tile([128, T], BF)
            tri_sb = cpool.tile([128, 256], BF)
            xT_sb = big.tile([128, KC, T], BF)
            wqk_sb = big.tile([128, KC, 2 * DG], BF)
            wv_sb = big.tile([128, KC, DG], BF)
            wout_sb = big.tile([128, 2, D], BF)
            # first q/k matmuls need wqk + xT chunk 0; order DMAs accordingly
            nc.sync.dma_start(out=wqk_sb[:], in_=wqk_d)
            nc.sync.dma_start(out=xT_sb[:, :, 0:512], in_=xT_d[:, :, 0:512])
            nc.sync.dma_start(out=wv_sb[:], in_=wv_d)
            nc.sync.dma_start(out=cos_sb[:], in_=cos_d)
            nc.sync.dma_start(out=sin_sb[:], in_=sin_d)
            nc.sync.dma_start(
                out=xT_sb[:, :, 512:1024], in_=xT_d[:, :, 512:1024])
            nc.sync.dma_start(out=tri_sb[:], in_=tri_d)
            nc.sync.dma_start(
                out=xT_sb[:, :, 1024:1536], in_=xT_d[:, :, 1024:1536])
            nc.sync.dma_start(
                out=xT_sb[:, :, 1536:2048], in_=xT_d[:, :, 1536:2048])
            nc.sync.dma_start(out=wout_sb[:], in_=wout_d)

            # PE warm-up: dummy matmuls fill the DMA lead-in so HAM unthrottles
            warm_sb = cpool.tile([128, 256], BF)
            nc.vector.memset(warm_sb[:].bitcast(F32), 0.0)
            for w in range(28):
                wp = ps_mm.tile([128, 512], F32, tag="mm")
                nc.tensor.matmul(
                    wp[:, 0:256], lhsT=warm_sb[:, 0:128], rhs=warm_sb[:],
                    start=True, stop=True,
                )

            # qkT_sb m-index: 0,1 = q head-pairs (0,1),(2,3); 2,3 = k pairs
            qkT_sb = big.tile([128, 4, T], BF)
            # HD+2 so the ones columns pair up into one f32 lane for memset:
            # 0x3F803F80 == two bf16 1.0s
            v_sb = big.tile([128, TT, HPG, HD + 2], BF)
            ones_f32 = float(np.frombuffer(
                np.uint32(0x3F803F80).tobytes(), dtype=np.float32)[0])
            nc.vector.memset(
                v_sb[:, :, :, HD:HD + 2].bitcast(F32), ones_f32)
            outT_sb = big.tile([128, 2, T], BF)

            def qk_unit(n, m, qkr):
                ns = slice(n * 512, (n + 1) * 512)
                ps = ps_mm.tile([128, 512], F32, tag="mm")
                for k in range(KC):
                    nc.tensor.matmul(
                        ps[:],
                        lhsT=wqk_sb[:, k, m * 128:(m + 1) * 128],
                        rhs=xT_sb[:, k, ns],
                        start=(k == 0),
                        stop=(k == KC - 1),
                    )
                nc.vector.tensor_copy(qkr[:, m, :], ps[:])

            def rope_finish(n, qkr):
                # partner swap p <-> p^16 via SBUF->SBUF DMA (frees DVE)
                ns = slice(n * 512, (n + 1) * 512)
                swp = ropep.tile([128, 4, 512], BF, tag="swp")
                for q in range(4):
                    nc.sync.dma_start(
                        out=swp[32 * q + 16:32 * q + 32], in_=qkr[32 * q:32 * q + 16])
                    nc.sync.dma_start(
                        out=swp[32 * q:32 * q + 16], in_=qkr[32 * q + 16:32 * q + 32])
                for m in range(4):
                    nc.vector.tensor_mul(qkr[:, m, :], qkr[:, m, :], cos_sb[:, ns])
                    nc.vector.tensor_mul(swp[:, m, :], swp[:, m, :], sin_sb[:, ns])
                    nc.vector.tensor_add(
                        qkT_sb[:, m, ns], qkr[:, m, :], swp[:, m, :])

            def v_unit(n, j):
                ps = ps_mm.tile([128, 256], F32, tag="mm")
                for k in range(KC):
                    nc.tensor.matmul(
                        ps[:],
                        lhsT=xT_sb[:, k, j * 128:(j + 1) * 128],
                        rhs=wv_sb[:, k, :],
                        start=(k == 0),
                        stop=(k == KC - 1),
                    )
                nc.vector.tensor_copy(
                    v_sb[:, j, :, 0:HD], ps[:].rearrange("p (h d) -> p h d", h=HPG)
                )

            def qkv_units(n):
                """Filler units (closures) for projecting chunk n."""
                qkr = [None]

                def qk_closure(m):
                    def f():
                        if m == 0:
                            qkr_t = ropep.tile([128, 4, 512], BF, tag="qkr")
                            qkr[0] = qkr_t
                        qk_unit(n, m, qkr[0])
                        if m == 3:
                            rope_finish(n, qkr[0])
                    return f
                return [qk_closure(m) for m in range(4)] + \
                       [lambda j=j: v_unit(n, j) for j in range(4 * n, 4 * n + 4)]

            def proj_unit(t, nh):
                ps = ps_mm.tile([128, 512], F32, tag="mm")
                for c in range(2):
                    nc.tensor.matmul(
                        ps[:],
                        lhsT=outT_sb[:, c, t * 128:(t + 1) * 128],
                        rhs=wout_sb[:, c, nh * 512:(nh + 1) * 512],
                        start=(c == 0),
                        stop=(c == 1),
                    )
                ot = outp.tile([128, 512], BF, tag="ot")
                if (t + nh) % 2 == 0:
                    nc.scalar.copy(out=ot[:], in_=ps[:])
                else:
                    nc.vector.tensor_copy(ot[:], ps[:])
                nc.sync.dma_start(
                    out=out_d[t * 128:(t + 1) * 128, nh * 512:(nh + 1) * 512],
                    in_=ot[:],
                )

            def proj_units(g):
                return [lambda t=t, nh=nh: proj_unit(t, nh)
                        for t in range(4 * g, 4 * g + 4) for nh in range(2)]

            fill_q = []

            def pump(k=1):
                for _ in range(k):
                    if fill_q:
                        fill_q.pop(0)()

            def attn_group(g, pat):
                """pat(step) -> how many filler units to pump after that step."""
                step = [0]
                for hp in range(2):
                    qm, km = hp, 2 + hp
                    pv0 = ps_pv0.tile([65, 512], F32, tag="pv0")
                    pv1 = ps_pv1.tile([65, 512], F32, tag="pv1")
                    jmax = 4 * g + 3
                    # diag blocks first so the causal masks (and their engine
                    # hops) sit early in the stream, not on the tail chain
                    j_order = list(range(4 * g, 4 * g + 4)) + list(range(4 * g))
                    exs = {}

                    def pv_step(idx):
                        j = j_order[idx]
                        ex = exs.pop(j)
                        d = j - 4 * g
                        nstart = 128 * d if d > 0 else 0
                        ncols = 512 - nstart
                        for half, pv in ((0, pv0), (1, pv1)):
                            nc.tensor.matmul(
                                pv[:, nstart:512],
                                lhsT=v_sb[:, j, 2 * hp + half, 0:HD + 1],
                                rhs=ex[:, half * 512:half * 512 + ncols],
                                start=(idx == 0),
                                stop=(idx == jmax),
                            )

                    for idx in range(jmax + 1):
                        j = j_order[idx]
                        d = j - 4 * g
                        nstart = 128 * d if d > 0 else 0
                        ncols = 512 - nstart
                        ex = expp.tile([128, 1024], BF, tag="ex")
                        exs[j] = ex
                        # two heads' score matmuls packed into one PE pass
                        # (row groups 0-1 / 2-3 run concurrently)
                        sc = ps_sc.tile([128, 1024], F32, tag="sc")
                        for half in range(2):
                            pb = 64 * half
                            nc.tensor.matmul(
                                sc[:, half * 512:half * 512 + ncols],
                                lhsT=qkT_sb[pb:pb + 64, km, j * 128:(j + 1) * 128],
                                rhs=qkT_sb[pb:pb + 64, qm,
                                           g * 512 + nstart:(g + 1) * 512],
                                start=True,
                                stop=True,
                            )
                        if ncols == 512:
                            nc.scalar.activation(
                                ex[:], sc[:],
                                mybir.ActivationFunctionType.Exp, scale=0.125,
                            )
                        else:
                            exv = ex[:].rearrange("p (u c) -> p u c", u=2)[:, :, 0:ncols]
                            scv = sc[:].rearrange("p (u c) -> p u c", u=2)[:, :, 0:ncols]
                            nc.scalar.activation(
                                exv, scv, mybir.ActivationFunctionType.Exp, scale=0.125,
                            )
                        if d >= 0:
                            # causal mask on the diagonal 128x128 of both heads
                            exm = ex[:].rearrange("p (u c) -> p u c", u=2)[:, :, 0:128]
                            trm = tri_sb[:].rearrange("p (u c) -> p u c", u=2)
                            nc.vector.tensor_mul(exm, exm, trm)
                        if idx > 0:
                            pv_step(idx - 1)
                        pump(pat(step[0]))
                        step[0] += 1
                    pv_step(jmax)
                    pump(2)

                    for half, pv in ((0, pv0), (1, pv1)):
                        pb = 64 * half
                        # reciprocal_approx_fast misreads PSUM operands; den
                        # must be staged through SBUF first
                        den = work.tile([1, 512], F32, tag="den")
                        nc.vector.tensor_copy(den[:], pv[64:65, :])
                        rec = work.tile([1, 512], F32, tag="rec")
                        nc.vector.reciprocal_approx_fast(rec[:], den[:])
                        recb = work.tile([64, 512], F32, tag="recb")
                        nc.gpsimd.partition_broadcast(recb[:], rec[0:1, :], channels=64)
                        nc.vector.tensor_mul(
                            outT_sb[pb:pb + 64, hp, g * 512:(g + 1) * 512],
                            pv[0:64, :],
                            recb[:],
                        )

            # lead-in: project chunk 0 densely, then attention with filler
            for u in qkv_units(0):
                u()
            fill_q.extend(qkv_units(1))
            attn_group(0, lambda s: 1)            # 8 steps, 8 units
            fill_q.extend(qkv_units(2))
            attn_group(1, lambda s: s % 2)        # 16 steps, 8 units
            fill_q.extend(qkv_units(3))
            fill_q.extend(proj_units(0))
            attn_group(2, lambda s: 1 if s % 3 != 2 else 0)  # 24 steps, 16 units
            fill_q.extend(proj_units(1))
            fill_q.extend(proj_units(2))
            attn_group(3, lambda s: (s + 1) % 2)  # 32 steps, 16 units
            pump(len(fill_q))
            for u in proj_units(3):
                u()

            if DEBUG_DUMPS:
                dq = nc.dram_tensor("dbg_qk", [128, 4, T], BF, kind="ExternalOutput").ap()
                dv = nc.dram_tensor("dbg_v", [128, TT, HPG, HD + 2], BF, kind="ExternalOutput").ap()
                do = nc.dram_tensor("dbg_outT", [128, 2, T], BF, kind="ExternalOutput").ap()
                nc.sync.dma_start(out=dq[:], in_=qkT_sb[:])
                nc.sync.dma_start(out=dv[:], in_=v_sb[:])
                nc.sync.dma_start(out=do[:], in_=outT_sb[:])

    nc.compile()
    return nc


def _qk_perm():
    """hd permutation for q/k columns: RoPE pair j -> (e,o) rows 16-interleaved
    so the swap stays within 32-partition quadrants (p <-> p^16)."""
    perm = np.empty(HD, dtype=np.int64)
    for p in range(HD):
        q32, i = divmod(p, 32)
        j = 16 * q32 + (i % 16)
        perm[p] = 2 * j + (1 if i >= 16 else 0)
    return perm


def _prepare_shards(x, w_qkv, w_out, freqs_cos, freqs_sin):
    perm = _qk_perm()
    cosT = np.ascontiguousarray(freqs_cos.T)  # [32, T]
    sinT = np.ascontiguousarray(freqs_sin.T)
    # row p of a 64-row head block: pair j = 16*(p//32 % 2) + p%16, sign -/+ for e/o
    cos128 = np.empty((128, T), dtype=np.float32)
    sin128s = np.empty((128, T), dtype=np.float32)
    for p in range(128):
        ph = p % 64
        q32, i = divmod(ph, 32)
        j = 16 * q32 + (i % 16)
        cos128[p] = cosT[j]
        sin128s[p] = sinT[j] * (-1.0 if i < 16 else 1.0)
    kk, qq = np.meshgrid(np.arange(128), np.arange(128), indexing="ij")
    tri = (kk <= qq).astype(np.float32)
    tri2 = np.concatenate([tri, tri], axis=1)  # [128, 256] for both heads

    w3 = w_qkv.reshape(D, 3, H, HD)
    in_maps = []
    for core in range(8):
        b, g = divmod(core, G)
        heads = np.arange(g * HPG, (g + 1) * HPG)
        wq = w3[:, 0, heads][:, :, perm].reshape(D, DG)
        wk = w3[:, 1, heads][:, :, perm].reshape(D, DG)
        wqk = np.ascontiguousarray(np.concatenate([wq, wk], axis=1))
        wv = np.ascontiguousarray(w3[:, 2, heads].reshape(D, DG))
        wo = np.ascontiguousarray(w_out.reshape(H, HD, D)[heads].reshape(DG, D))
        def sb_layout(a, kc=KC):
            # [128*kc, F] -> [128, kc, F] with partition-major contiguity
            return np.ascontiguousarray(
                a.reshape(kc, 128, -1).transpose(1, 0, 2)).astype(NPBF)
        in_maps.append({
            "xT": sb_layout(x[b].T),
            "wqk": sb_layout(wqk),
            "wv": sb_layout(wv),
            "wout": sb_layout(wo, kc=2),
            "cos128": cos128.astype(NPBF),
            "sin128s": sin128s.astype(NPBF),
            "tri2": tri2.astype(NPBF),
        })
    return in_maps


def _run(in_maps, **kw):
    if "nc" not in _CACHE:
        _CACHE["nc"] = _build()
    return run_bass_kernel_spmd(_CACHE["nc"], in_maps, core_ids=list(range(8)), **kw)


def kernel(x, w_qkv, w_out, freqs_cos, freqs_sin):
    x = np.asarray(x, dtype=np.float32)
    w_qkv = np.asarray(w_qkv, dtype=np.float32)
    w_out = np.asarray(w_out, dtype=np.float32)
    freqs_cos = np.asarray(freqs_cos, dtype=np.float32)
    freqs_sin = np.asarray(freqs_sin, dtype=np.float32)

    in_maps = _prepare_shards(x, w_qkv, w_out, freqs_cos, freqs_sin)
    res = _run(in_maps)
    out = np.zeros((B, T, D), dtype=np.float64)
    for core in range(8):
        out[core // G] += res.results[core]["out"].astype(np.float64)
    return out.astype(np.float32)


# revision 18
# speedup vs baseline: 1.4308x; 1.1213x over previous
"""Causal self-attention (B=2,T=2048,D=1024,H=16,HD=64) + RoPE on 8 TRN2 NeuronCores.

Sharding: core = b*4 + g  (b: batch, g: head-group of 4 heads).
Each core computes QKV projection for its 4 heads, causal attention, and a
partial out-projection (rank-256 contribution). Host sums the 4 partials per
batch (the "all-reduce after out_proj").

v2: all matmul operands bf16 (same PE streaming rate as fp32r, but FWL weight
loads, half DMA bytes, 2x DVE modes). RoPE partner-swap moved off DVE onto
SBUF->SBUF DMA; causal masks on gpsimd; exp on ACT (the only exp engine).
Attention j-steps are software-pipelined (scores j+1 issued before PV j) and
interleaved with QKV/out-proj matmul "filler" so the PE never idles long
enough for HAM to re-throttle to half clock.
"""
import numpy as np
import ml_dtypes

import concourse.bass as bass
import concourse.mybir as mybir
from concourse import bacc
from concourse.tile import TileContext
from concourse.bass_utils import run_bass_kernel_spmd

B, T, D, H = 2, 2048, 1024, 16
HD = D // H            # 64
G = 4                  # head groups (tensor-parallel factor)
HPG = H // G           # 4 heads per group
DG = HPG * HD          # 256 head-dims per group
KC = D // 128          # 8 contraction chunks for D
NT = T // 512          # 4 T-chunks of 512
TT = T // 128          # 16 T-tiles of 128
F32 = mybir.dt.float32
BF = mybir.dt.bfloat16
NPBF = ml_dtypes.bfloat16

_CACHE = {}
DEBUG_DUMPS = False


def _build():
    nc = bacc.Bacc("TRN2", target_bir_lowering=False, debug=False, num_devices=8)

    xT_d = nc.dram_tensor("xT", [128, KC, T], BF, kind="ExternalInput").ap()
    wqk_d = nc.dram_tensor("wqk", [128, KC, 2 * DG], BF, kind="ExternalInput").ap()
    wv_d = nc.dram_tensor("wv", [128, KC, DG], BF, kind="ExternalInput").ap()
    wout_d = nc.dram_tensor("wout", [128, 2, D], BF, kind="ExternalInput").ap()
    cos_d = nc.dram_tensor("cos128", [128, T], BF, kind="ExternalInput").ap()
    sin_d = nc.dram_tensor("sin128s", [128, T], BF, kind="ExternalInput").ap()
    tri_d = nc.dram_tensor("tri2", [128, 256], BF, kind="ExternalInput").ap()
    out_d = nc.dram_tensor("out", [T, D], BF, kind="ExternalOutput").ap()

    with TileContext(nc) as tc:
        with (
            tc.tile_pool(name="const", bufs=1) as cpool,
            tc.tile_pool(name="big", bufs=1) as big,
            tc.tile_pool(name="rope", bufs=2) as ropep,
            tc.tile_pool(name="work", bufs=2) as work,
            tc.tile_pool(name="expp", bufs=3) as expp,
            tc.tile_pool(name="outp", bufs=2) as outp,
            tc.tile_pool(name="ps_mm", bufs=2, space="PSUM") as ps_mm,
            tc.tile_pool(name="ps_sc", bufs=2, space="PSUM") as ps_sc,
            tc.tile_pool(name="ps_pv0", bufs=1, space="PSUM") as ps_pv0,
            tc.tile_pool(name="ps_pv1", bufs=1, space="PSUM") as ps_pv1,
        ):
            cos_sb = cpool.tile([128, T], BF)
            sin_sb = cpool.tile([128, T], BF)
            tri_sb = cpool.tile([128, 256], BF)
            xT_sb = big.tile([128, KC, T], BF)
            wqk_sb = big.tile([128, KC, 2 * DG], BF)
            wv_sb = big.tile([128, KC, DG], BF)
            wout_sb = big.tile([128, 2, D], BF)
            # first q/k matmuls need wqk + xT chunk 0; order DMAs accordingly
            nc.sync.dma_start(out=wqk_sb[:], in_=wqk_d)
            nc.sync.dma_start(out=xT_sb[:, :, 0:512], in_=xT_d[:, :, 0:512])
            nc.sync.dma_start(out=wv_sb[:], in_=wv_d)
            nc.sync.dma_start(out=cos_sb[:], in_=cos_d)
            nc.sync.dma_start(out=sin_sb[:], in_=sin_d)
            nc.sync.dma_start(
                out=xT_sb[:, :, 512:1024], in_=xT_d[:, :, 512:1024])
            nc.sync.dma_start(out=tri_sb[:], in_=tri_d)
            nc.sync.dma_start(
                out=xT_sb[:, :, 1024:1536], in_=xT_d[:, :, 1024:1536])
            nc.sync.dma_start(
                out=xT_sb[:, :, 1536:2048], in_=xT_d[:, :, 1536:2048])
            nc.sync.dma_start(out=wout_sb[:], in_=wout_d)

            # PE warm-up: dummy matmuls fill the DMA lead-in so HAM unthrottles
            warm_sb = cpool.tile([128, 256], BF)
            nc.vector.memset(warm_sb[:].bitcast(F32), 0.0)
            for w in range(28):
                wp = ps_mm.tile([128, 512], F32, tag="mm")
                nc.tensor.matmul(
                    wp[:, 0:256], lhsT=warm_sb[:, 0:128], rhs=warm_sb[:],
                    start=True, stop=True,
                )

            # qkT_sb m-index: 0,1 = q head-pairs (0,1),(2,3); 2,3 = k pairs
            qkT_sb = big.tile([128, 4, T], BF)
            # HD+2 so the ones columns pair up into one f32 lane for memset:
            # 0x3F803F80 == two bf16 1.0s
            v_sb = big.tile([128, TT, HPG, HD + 2], BF)
            ones_f32 = float(np.frombuffer(
                np.uint32(0x3F803F80).tobytes(), dtype=np.float32)[0])
            nc.vector.memset(
                v_sb[:, :, :, HD:HD + 2].bitcast(F32), ones_f32)
            outT_sb = big.tile([128, 2, T], BF)

            def qk_unit(n, m, t1, uu):
                # rot(x)[p] = x[p]*cos[p] + x[p^16]*sin_signed[p]; with
                # sin2[p] = sin_signed[p^16] pre-applied, swap(x*sin2) gives
                # the second term -- no raw-copy staging needed.
                ns = slice(n * 512, (n + 1) * 512)
                ps = ps_mm.tile([128, 512], F32, tag="mm")
                for k in range(KC):
                    nc.tensor.matmul(
                        ps[:],
                        lhsT=wqk_sb[:, k, m * 128:(m + 1) * 128],
                        rhs=xT_sb[:, k, ns],
                        start=(k == 0),
                        stop=(k == KC - 1),
                    )
                nc.vector.tensor_mul(uu[:, m, :], ps[:], sin_sb[:, ns])
                nc.vector.tensor_mul(t1[:, m, :], ps[:], cos_sb[:, ns])

            def rope_finish(n, t1, uu):
                # partner swap p <-> p^16 via SBUF->SBUF DMA (frees DVE)
                ns = slice(n * 512, (n + 1) * 512)
                us = ropep.tile([128, 4, 512], BF, tag="us")
                for q in range(4):
                    nc.sync.dma_start(
                        out=us[32 * q + 16:32 * q + 32], in_=uu[32 * q:32 * q + 16])
                    nc.sync.dma_start(
                        out=us[32 * q:32 * q + 16], in_=uu[32 * q + 16:32 * q + 32])
                nc.vector.tensor_add(qkT_sb[:, :, ns], t1[:], us[:])

            def v_unit(n, j):
                ps = ps_mm.tile([128, 256], F32, tag="mm")
                for k in range(KC):
                    nc.tensor.matmul(
                        ps[:],
                        lhsT=xT_sb[:, k, j * 128:(j + 1) * 128],
                        rhs=wv_sb[:, k, :],
                        start=(k == 0),
                        stop=(k == KC - 1),
                    )
                nc.vector.tensor_copy(
                    v_sb[:, j, :, 0:HD], ps[:].rearrange("p (h d) -> p h d", h=HPG)
                )

            def qkv_units(n):
                """Filler units (closures) for projecting chunk n."""
                st = [None, None]

                def qk_closure(m):
                    def f():
                        if m == 0:
                            t1_t = ropep.tile([128, 4, 512], BF, tag="t1")
                            uu_t = ropep.tile([128, 4, 512], BF, tag="uu")
                            st[0], st[1] = t1_t, uu_t
                        qk_unit(n, m, st[0], st[1])
                        if m == 3:
                            rope_finish(n, st[0], st[1])
                    return f
                return [qk_closure(m) for m in range(4)] + \
                       [lambda j=j: v_unit(n, j) for j in range(4 * n, 4 * n + 4)]

            def proj_unit(t, nh):
                ps = ps_mm.tile([128, 512], F32, tag="mm")
                for c in range(2):
                    nc.tensor.matmul(
                        ps[:],
                        lhsT=outT_sb[:, c, t * 128:(t + 1) * 128],
                        rhs=wout_sb[:, c, nh * 512:(nh + 1) * 512],
                        start=(c == 0),
                        stop=(c == 1),
                    )
                ot = outp.tile([128, 512], BF, tag="ot")
                if (t + nh) % 2 == 0:
                    nc.scalar.copy(out=ot[:], in_=ps[:])
                else:
                    nc.vector.tensor_copy(ot[:], ps[:])
                nc.sync.dma_start(
                    out=out_d[t * 128:(t + 1) * 128, nh * 512:(nh + 1) * 512],
                    in_=ot[:],
                )

            def proj_units(g):
                return [lambda t=t, nh=nh: proj_unit(t, nh)
                        for t in range(4 * g, 4 * g + 4) for nh in range(2)]

            fill_q = []

            def pump(k=1):
                for _ in range(k):
                    if fill_q:
                        fill_q.pop(0)()

            def attn_group(g, pat):
                """pat(step) -> how many filler units to pump after that step."""
                step = [0]
                for hp in range(2):
                    qm, km = hp, 2 + hp
                    pv0 = ps_pv0.tile([65, 512], F32, tag="pv0")
                    pv1 = ps_pv1.tile([65, 512], F32, tag="pv1")
                    jmax = 4 * g + 3
                    # diag blocks first so the causal masks (and their engine
                    # hops) sit early in the stream, not on the tail chain
                    j_order = list(range(4 * g, 4 * g + 4)) + list(range(4 * g))
                    exs = {}

                    def pv_step(idx):
                        j = j_order[idx]
                        ex = exs.pop(j)
                        d = j - 4 * g
                        nstart = 128 * d if d > 0 else 0
                        ncols = 512 - nstart
                        for half, pv in ((0, pv0), (1, pv1)):
                            nc.tensor.matmul(
                                pv[:, nstart:512],
                                lhsT=v_sb[:, j, 2 * hp + half, 0:HD + 1],
                                rhs=ex[:, half * 512:half * 512 + ncols],
                                start=(idx == 0),
                                stop=(idx == jmax),
                            )

                    for idx in range(jmax + 1):
                        j = j_order[idx]
                        d = j - 4 * g
                        nstart = 128 * d if d > 0 else 0
                        ncols = 512 - nstart
                        ex = expp.tile([128, 1024], BF, tag="ex")
                        exs[j] = ex
                        # two heads' score matmuls packed into one PE pass
                        # (row groups 0-1 / 2-3 run concurrently)
                        sc = ps_sc.tile([128, 1024], F32, tag="sc")
                        for half in range(2):
                            pb = 64 * half
                            nc.tensor.matmul(
                                sc[:, half * 512:half * 512 + ncols],
                                lhsT=qkT_sb[pb:pb + 64, km, j * 128:(j + 1) * 128],
                                rhs=qkT_sb[pb:pb + 64, qm,
                                           g * 512 + nstart:(g + 1) * 512],
                                start=True,
                                stop=True,
                            )
                        if ncols == 512:
                            nc.scalar.activation(
                                ex[:], sc[:],
                                mybir.ActivationFunctionType.Exp, scale=0.125,
                            )
                        else:
                            exv = ex[:].rearrange("p (u c) -> p u c", u=2)[:, :, 0:ncols]
                            scv = sc[:].rearrange("p (u c) -> p u c", u=2)[:, :, 0:ncols]
                            nc.scalar.activation(
                                exv, scv, mybir.ActivationFunctionType.Exp, scale=0.125,
                            )
                        if d >= 0:
                            # causal mask on the diagonal 128x128 of both heads
                            exm = ex[:].rearrange("p (u c) -> p u c", u=2)[:, :, 0:128]
                            trm = tri_sb[:].rearrange("p (u c) -> p u c", u=2)
                            nc.vector.tensor_mul(exm, exm, trm)
                        if idx > 0:
                            pv_step(idx - 1)
                        pump(pat(step[0]))
                        step[0] += 1
                    pv_step(jmax)
                    pump(2)

                    for half, pv in ((0, pv0), (1, pv1)):
                        pb = 64 * half
                        # reciprocal_approx_fast misreads PSUM operands; den
                        # must be staged through SBUF first
                        den = work.tile([1, 512], F32, tag="den")
                        nc.vector.tensor_copy(den[:], pv[64:65, :])
                        rec = work.tile([1, 512], F32, tag="rec")
                        nc.vector.reciprocal_approx_fast(rec[:], den[:])
                        recb = work.tile([64, 512], F32, tag="recb")
                        nc.gpsimd.partition_broadcast(recb[:], rec[0:1, :], channels=64)
                        nc.vector.tensor_mul(
                            outT_sb[pb:pb + 64, hp, g * 512:(g + 1) * 512],
                            pv[0:64, :],
                            recb[:],
                        )

            # lead-in: project chunk 0 densely, then attention with filler
            for u in qkv_units(0):
                u()
            fill_q.extend(qkv_units(1))
            attn_group(0, lambda s: 1)            # 8 steps, 8 units
            fill_q.extend(qkv_units(2))
            attn_group(1, lambda s: s % 2)        # 16 steps, 8 units
            fill_q.extend(qkv_units(3))
            fill_q.extend(proj_units(0))
            attn_group(2, lambda s: 1 if s < 16 else 0)  # 24 steps, 16 units
            fill_q.extend(proj_units(1))
            fill_q.extend(proj_units(2))
            attn_group(3, lambda s: (s + 1) % 2)  # 32 steps, 16 units
            pump(len(fill_q))
            for u in proj_units(3):
                u()

            if DEBUG_DUMPS:
                dq = nc.dram_tensor("dbg_qk", [128, 4, T], BF, kind="ExternalOutput").ap()
                dv = nc.dram_tensor("dbg_v", [128, TT, HPG, HD + 2], BF, kind="ExternalOutput").ap()
                do = nc.dram_tensor("dbg_outT", [128, 2, T], BF, kind="ExternalOutput").ap()
                nc.sync.dma_start(out=dq[:], in_=qkT_sb[:])
                nc.sync.dma_start(out=dv[:], in_=v_sb[:])
                nc.sync.dma_start(out=do[:], in_=outT_sb[:])

    nc.compile()
    return nc


def _qk_perm():
    """hd permutation for q/k columns: RoPE pair j -> (e,o) rows 16-interleaved
    so the swap stays within 32-partition quadrants (p <-> p^16)."""
    perm = np.empty(HD, dtype=np.int64)
    for p in range(HD):
        q32, i = divmod(p, 32)
        j = 16 * q32 + (i % 16)
        perm[p] = 2 * j + (1 if i >= 16 else 0)
    return perm


def _prepare_shards(x, w_qkv, w_out, freqs_cos, freqs_sin):
    perm = _qk_perm()
    cosT = np.ascontiguousarray(freqs_cos.T)  # [32, T]
    sinT = np.ascontiguousarray(freqs_sin.T)
    # row p of a 64-row head block: pair j = 16*(p//32 % 2) + p%16, sign -/+ for e/o
    cos128 = np.empty((128, T), dtype=np.float32)
    sin128s = np.empty((128, T), dtype=np.float32)
    for p in range(128):
        ph = p % 64
        q32, i = divmod(ph, 32)
        j = 16 * q32 + (i % 16)
        cos128[p] = cosT[j]
        sin128s[p] = sinT[j] * (-1.0 if i < 16 else 1.0)
    # pre-shuffled sin: device computes u = raw*sin2, then swap(u) equals
    # raw[p^16]*sin_signed[p]
    swap16 = np.array([(p // 32) * 32 + ((p % 32) + 16) % 32 for p in range(128)])
    sin2 = sin128s[swap16]
    kk, qq = np.meshgrid(np.arange(128), np.arange(128), indexing="ij")
    tri = (kk <= qq).astype(np.float32)
    tri2 = np.concatenate([tri, tri], axis=1)  # [128, 256] for both heads

    w3 = w_qkv.reshape(D, 3, H, HD)
    in_maps = []
    for core in range(8):
        b, g = divmod(core, G)
        heads = np.arange(g * HPG, (g + 1) * HPG)
        wq = w3[:, 0, heads][:, :, perm].reshape(D, DG)
        wk = w3[:, 1, heads][:, :, perm].reshape(D, DG)
        wqk = np.ascontiguousarray(np.concatenate([wq, wk], axis=1))
        wv = np.ascontiguousarray(w3[:, 2, heads].reshape(D, DG))
        wo = np.ascontiguousarray(w_out.reshape(H, HD, D)[heads].reshape(DG, D))
        def sb_layout(a, kc=KC):
            # [128*kc, F] -> [128, kc, F] with partition-major contiguity
            return np.ascontiguousarray(
                a.reshape(kc, 128, -1).transpose(1, 0, 2)).astype(NPBF)
        in_maps.append({
            "xT": sb_layout(x[b].T),
            "wqk": sb_layout(wqk),
            "wv": sb_layout(wv),
            "wout": sb_layout(wo, kc=2),
            "cos128": cos128.astype(NPBF),
            "sin128s": sin2.astype(NPBF),
            "tri2": tri2.astype(NPBF),
        })
    return in_maps


def _run(in_maps, **kw):
    if "nc" not in _CACHE:
        _CACHE["nc"] = _build()
    return run_bass_kernel_spmd(_CACHE["nc"], in_maps, core_ids=list(range(8)), **kw)


def kernel(x, w_qkv, w_out, freqs_cos, freqs_sin):
    x = np.asarray(x, dtype=np.float32)
    w_qkv = np.asarray(w_qkv, dtype=np.float32)
    w_out = np.asarray(w_out, dtype=np.float32)
    freqs_cos = np.asarray(freqs_cos, dtype=np.float32)
    freqs_sin = np.asarray(freqs_sin, dtype=np.float32)

    in_maps = _prepare_shards(x, w_qkv, w_out, freqs_cos, freqs_sin)
    res = _run(in_maps)
    out = np.zeros((B, T, D), dtype=np.float64)
    for core in range(8):
        out[core // G] += res.results[core]["out"].astype(np.float64)
    return out.astype(np.float32)


# revision 24
# speedup vs baseline: 1.4741x; 1.0303x over previous
"""Causal self-attention (B=2,T=2048,D=1024,H=16,HD=64) + RoPE on 8 TRN2 NeuronCores.

Sharding: core = b*4 + g  (b: batch, g: head-group of 4 heads).
Each core computes QKV projection for its 4 heads, causal attention, and a
partial out-projection (rank-256 contribution). Host sums the 4 partials per
batch (the "all-reduce after out_proj").

v2: all matmul operands bf16 (same PE streaming rate as fp32r, but FWL weight
loads, half DMA bytes, 2x DVE modes). RoPE partner-swap moved off DVE onto
SBUF->SBUF DMA; causal masks on gpsimd; exp on ACT (the only exp engine).
Attention j-steps are software-pipelined (scores j+1 issued before PV j) and
interleaved with QKV/out-proj matmul "filler" so the PE never idles long
enough for HAM to re-throttle to half clock.
"""
import numpy as np
import ml_dtypes

import concourse.bass as bass
import concourse.mybir as mybir
from concourse import bacc
from concourse.tile import TileContext
from concourse.bass_utils import run_bass_kernel_spmd

B, T, D, H = 2, 2048, 1024, 16
HD = D // H            # 64
G = 4                  # head groups (tensor-parallel factor)
HPG = H // G           # 4 heads per group
DG = HPG * HD          # 256 head-dims per group
KC = D // 128          # 8 contraction chunks for D
NT = T // 512          # 4 T-chunks of 512
TT = T // 128          # 16 T-tiles of 128
F32 = mybir.dt.float32
BF = mybir.dt.bfloat16
NPBF = ml_dtypes.bfloat16

_CACHE = {}
DEBUG_DUMPS = False


def _build():
    nc = bacc.Bacc("TRN2", target_bir_lowering=False, debug=False, num_devices=8)

    xT_d = nc.dram_tensor("xT", [128, KC, T], BF, kind="ExternalInput").ap()
    wqk_d = nc.dram_tensor("wqk", [128, KC, 2 * DG], BF, kind="ExternalInput").ap()
    wv_d = nc.dram_tensor("wv", [128, KC, DG], BF, kind="ExternalInput").ap()
    wout_d = nc.dram_tensor("wout", [128, 2, D], BF, kind="ExternalInput").ap()
    cos_d = nc.dram_tensor("cos128", [128, T], BF, kind="ExternalInput").ap()
    sin_d = nc.dram_tensor("sin128s", [128, T], BF, kind="ExternalInput").ap()
    tri_d = nc.dram_tensor("tri2", [128, 256], BF, kind="ExternalInput").ap()
    out_d = nc.dram_tensor("out", [T, D], BF, kind="ExternalOutput").ap()

    with TileContext(nc) as tc:
        with (
            tc.tile_pool(name="const", bufs=1) as cpool,
            tc.tile_pool(name="big", bufs=1) as big,
            tc.tile_pool(name="rope", bufs=2) as ropep,
            tc.tile_pool(name="work", bufs=2) as work,
            tc.tile_pool(name="expp", bufs=3) as expp,
            tc.tile_pool(name="outp", bufs=2) as outp,
            tc.tile_pool(name="ps_mm", bufs=2, space="PSUM") as ps_mm,
            tc.tile_pool(name="ps_sc", bufs=2, space="PSUM") as ps_sc,
            tc.tile_pool(name="ps_pv0", bufs=1, space="PSUM") as ps_pv0,
            tc.tile_pool(name="ps_pv1", bufs=1, space="PSUM") as ps_pv1,
        ):
            cos_sb = cpool.tile([128, T], BF)
            sin_sb = cpool.tile([128, T], BF)
            tri_sb = cpool.tile([128, 256], BF)
            xT_sb = big.tile([128, KC, T], BF)
            wqk_sb = big.tile([128, KC, 2 * DG], BF)
            wv_sb = big.tile([128, KC, DG], BF)
            wout_sb = big.tile([128, 2, D], BF)
            # first q/k matmuls need wqk + xT chunk 0; order DMAs accordingly
            # only first-needed tensors upfront; xT chunks 1-3 and wout are
            # prefetched from inside earlier filler units so the HW DMA
            # queues don't round-robin everything to a late joint finish
            nc.sync.dma_start(out=wqk_sb[:], in_=wqk_d)
            nc.sync.dma_start(out=xT_sb[:, :, 0:512], in_=xT_d[:, :, 0:512])
            nc.sync.dma_start(out=wv_sb[:], in_=wv_d)
            nc.sync.dma_start(out=cos_sb[:], in_=cos_d)
            nc.sync.dma_start(out=sin_sb[:], in_=sin_d)
            nc.sync.dma_start(out=tri_sb[:], in_=tri_d)

            def stage_dma(n):
                if n < NT:
                    nc.sync.dma_start(
                        out=xT_sb[:, :, n * 512:(n + 1) * 512],
                        in_=xT_d[:, :, n * 512:(n + 1) * 512])
                if n == 2:
                    nc.sync.dma_start(out=wout_sb[:], in_=wout_d)

            # PE warm-up: dummy matmuls fill the DMA lead-in so HAM unthrottles
            warm_sb = cpool.tile([128, 256], BF)
            nc.vector.memset(warm_sb[:].bitcast(F32), 0.0)
            for w in range(45):
                wp = ps_mm.tile([128, 512], F32, tag="mm")
                nc.tensor.matmul(
                    wp[:, 0:256], lhsT=warm_sb[:, 0:128], rhs=warm_sb[:],
                    start=True, stop=True,
                )

            # qkT_sb m-index: 0,1 = q head-pairs (0,1),(2,3); 2,3 = k pairs
            qkT_sb = big.tile([128, 4, T], BF)
            # HD+2 so the ones columns pair up into one f32 lane for memset:
            # 0x3F803F80 == two bf16 1.0s
            v_sb = big.tile([128, TT, HPG, HD + 2], BF)
            ones_f32 = float(np.frombuffer(
                np.uint32(0x3F803F80).tobytes(), dtype=np.float32)[0])
            nc.vector.memset(
                v_sb[:, :, :, HD:HD + 2].bitcast(F32), ones_f32)
            outT_sb = big.tile([128, 2, T], BF)

            def qk_unit(n, m, t1, uu):
                # rot(x)[p] = x[p]*cos[p] + x[p^16]*sin_signed[p]; with
                # sin2[p] = sin_signed[p^16] pre-applied, swap(x*sin2) gives
                # the second term. Single PSUM reader (the copy) so the mm
                # bank releases fast for the next filler unit.
                ns = slice(n * 512, (n + 1) * 512)
                ps = ps_mm.tile([128, 512], F32, tag="mm")
                for k in range(KC):
                    nc.tensor.matmul(
                        ps[:],
                        lhsT=wqk_sb[:, k, m * 128:(m + 1) * 128],
                        rhs=xT_sb[:, k, ns],
                        start=(k == 0),
                        stop=(k == KC - 1),
                    )
                raw = ropep.tile([128, 512], BF, tag="raw")
                nc.vector.tensor_copy(raw[:], ps[:])
                nc.vector.tensor_mul(uu[:, m, :], raw[:], sin_sb[:, ns])
                nc.vector.tensor_mul(t1[:, m, :], raw[:], cos_sb[:, ns])

            def rope_finish(n, t1, uu):
                # partner swap p <-> p^16 via SBUF->SBUF DMA (frees DVE)
                ns = slice(n * 512, (n + 1) * 512)
                us = ropep.tile([128, 4, 512], BF, tag="us")
                for q in range(4):
                    nc.sync.dma_start(
                        out=us[32 * q + 16:32 * q + 32], in_=uu[32 * q:32 * q + 16])
                    nc.sync.dma_start(
                        out=us[32 * q:32 * q + 16], in_=uu[32 * q + 16:32 * q + 32])
                nc.vector.tensor_add(qkT_sb[:, :, ns], t1[:], us[:])

            def v_unit(n, j):
                ps = ps_mm.tile([128, 256], F32, tag="mm")
                for k in range(KC):
                    nc.tensor.matmul(
                        ps[:],
                        lhsT=xT_sb[:, k, j * 128:(j + 1) * 128],
                        rhs=wv_sb[:, k, :],
                        start=(k == 0),
                        stop=(k == KC - 1),
                    )
                nc.vector.tensor_copy(
                    v_sb[:, j, :, 0:HD], ps[:].rearrange("p (h d) -> p h d", h=HPG)
                )

            def qkv_units(n):
                """Filler units (closures) for projecting chunk n."""
                st = [None, None]

                def qk_closure(m):
                    def f():
                        if m == 0:
                            stage_dma(n + 1)
                            t1_t = ropep.tile([128, 4, 512], BF, tag="t1")
                            uu_t = ropep.tile([128, 4, 512], BF, tag="uu")
                            st[0], st[1] = t1_t, uu_t
                        qk_unit(n, m, st[0], st[1])
                        if m == 3:
                            rope_finish(n, st[0], st[1])
                    return f
                return [qk_closure(m) for m in range(4)] + \
                       [lambda j=j: v_unit(n, j) for j in range(4 * n, 4 * n + 4)]

            def proj_unit(t, nh):
                ps = ps_mm.tile([128, 512], F32, tag="mm")
                for c in range(2):
                    nc.tensor.matmul(
                        ps[:],
                        lhsT=outT_sb[:, c, t * 128:(t + 1) * 128],
                        rhs=wout_sb[:, c, nh * 512:(nh + 1) * 512],
                        start=(c == 0),
                        stop=(c == 1),
                    )
                ot = outp.tile([128, 512], BF, tag="ot")
                if (t + nh) % 2 == 0:
                    nc.scalar.copy(out=ot[:], in_=ps[:])
                else:
                    nc.vector.tensor_copy(ot[:], ps[:])
                nc.sync.dma_start(
                    out=out_d[t * 128:(t + 1) * 128, nh * 512:(nh + 1) * 512],
                    in_=ot[:],
                )

            def proj_units(g):
                return [lambda t=t, nh=nh: proj_unit(t, nh)
                        for t in range(4 * g, 4 * g + 4) for nh in range(2)]

            fill_q = []

            def pump(k=1):
                for _ in range(k):
                    if fill_q:
                        fill_q.pop(0)()

            def attn_group(g, pat, last=False):
                """pat(step) -> how many filler units to pump after that step."""
                step = [0]
                for hp in range(2):
                    qm, km = hp, 2 + hp
                    pv0 = ps_pv0.tile([65, 512], F32, tag="pv0")
                    pv1 = ps_pv1.tile([65, 512], F32, tag="pv1")
                    jmax = 4 * g + 3
                    # diag blocks first so the causal masks (and their engine
                    # hops) sit early in the stream, not on the tail chain
                    j_order = list(range(4 * g, 4 * g + 4)) + list(range(4 * g))
                    exs = {}

                    def pv_step(idx):
                        j = j_order[idx]
                        ex = exs.pop(j)
                        d = j - 4 * g
                        nstart = 128 * d if d > 0 else 0
                        ncols = 512 - nstart
                        for half, pv in ((0, pv0), (1, pv1)):
                            nc.tensor.matmul(
                                pv[:, nstart:512],
                                lhsT=v_sb[:, j, 2 * hp + half, 0:HD + 1],
                                rhs=ex[:, half * 512:half * 512 + ncols],
                                start=(idx == 0),
                                stop=(idx == jmax),
                            )

                    for idx in range(jmax + 1):
                        j = j_order[idx]
                        d = j - 4 * g
                        nstart = 128 * d if d > 0 else 0
                        ncols = 512 - nstart
                        ex = expp.tile([128, 1024], BF, tag="ex")
                        exs[j] = ex
                        # two heads' score matmuls packed into one PE pass
                        # (row groups 0-1 / 2-3 run concurrently)
                        sc = ps_sc.tile([128, 1024], F32, tag="sc")
                        for half in range(2):
                            pb = 64 * half
                            nc.tensor.matmul(
                                sc[:, half * 512:half * 512 + ncols],
                                lhsT=qkT_sb[pb:pb + 64, km, j * 128:(j + 1) * 128],
                                rhs=qkT_sb[pb:pb + 64, qm,
                                           g * 512 + nstart:(g + 1) * 512],
                                start=True,
                                stop=True,
                            )
                        if ncols == 512:
                            nc.scalar.activation(
                                ex[:], sc[:],
                                mybir.ActivationFunctionType.Exp, scale=0.125,
                            )
                        else:
                            exv = ex[:].rearrange("p (u c) -> p u c", u=2)[:, :, 0:ncols]
                            scv = sc[:].rearrange("p (u c) -> p u c", u=2)[:, :, 0:ncols]
                            nc.scalar.activation(
                                exv, scv, mybir.ActivationFunctionType.Exp, scale=0.125,
                            )
                        if d >= 0:
                            # causal mask on the diagonal 128x128 of both heads
                            exm = ex[:].rearrange("p (u c) -> p u c", u=2)[:, :, 0:128]
                            trm = tri_sb[:].rearrange("p (u c) -> p u c", u=2)
                            nc.vector.tensor_mul(exm, exm, trm)
                        if idx > 0:
                            pv_step(idx - 1)
                        pump(pat(step[0]))
                        step[0] += 1
                    pv_step(jmax)
                    if not (last and hp == 1):
                        pump(2)

                    for half, pv in ((0, pv0), (1, pv1)):
                        pb = 64 * half
                        # reciprocal_approx_fast misreads PSUM operands; den
                        # must be staged through SBUF first
                        den = work.tile([1, 512], F32, tag="den")
                        nc.vector.tensor_copy(den[:], pv[64:65, :])
                        rec = work.tile([1, 512], F32, tag="rec")
                        nc.vector.reciprocal_approx_fast(rec[:], den[:])
                        recb = work.tile([64, 512], F32, tag="recb")
                        nc.gpsimd.partition_broadcast(recb[:], rec[0:1, :], channels=64)
                        nc.vector.tensor_mul(
                            outT_sb[pb:pb + 64, hp, g * 512:(g + 1) * 512],
                            pv[0:64, :],
                            recb[:],
                        )

            # lead-in: project chunk 0 densely, then attention with filler
            for u in qkv_units(0):
                u()
            fill_q.extend(qkv_units(1))
            attn_group(0, lambda s: 1)            # 8 steps, 8 units
            fill_q.extend(qkv_units(2))
            attn_group(1, lambda s: s % 2)        # 16 steps, 8 units
            fill_q.extend(qkv_units(3))
            fill_q.extend(proj_units(0))
            attn_group(2, lambda s: 1 if s < 16 else 0)  # 24 steps, 16 units
            fill_q.extend(proj_units(1))
            fill_q.extend(proj_units(2))
            attn_group(3, lambda s: (s + 1) % 2, last=True)  # 32 steps, 16 units
            pump(len(fill_q))
            for u in proj_units(3):
                u()

            if DEBUG_DUMPS:
                dq = nc.dram_tensor("dbg_qk", [128, 4, T], BF, kind="ExternalOutput").ap()
                dv = nc.dram_tensor("dbg_v", [128, TT, HPG, HD + 2], BF, kind="ExternalOutput").ap()
                do = nc.dram_tensor("dbg_outT", [128, 2, T], BF, kind="ExternalOutput").ap()
                nc.sync.dma_start(out=dq[:], in_=qkT_sb[:])
                nc.sync.dma_start(out=dv[:], in_=v_sb[:])
                nc.sync.dma_start(out=do[:], in_=outT_sb[:])

    nc.compile()
    return nc


def _qk_perm():
    """hd permutation for q/k columns: RoPE pair j -> (e,o) rows 16-interleaved
    so the swap stays within 32-partition quadrants (p <-> p^16)."""
    perm = np.empty(HD, dtype=np.int64)
    for p in range(HD):
        q32, i = divmod(p, 32)
        j = 16 * q32 + (i % 16)
        perm[p] = 2 * j + (1 if i >= 16 else 0)
    return perm


def _prepare_shards(x, w_qkv, w_out, freqs_cos, freqs_sin):
    perm = _qk_perm()
    cosT = np.ascontiguousarray(freqs_cos.T)  # [32, T]
    sinT = np.ascontiguousarray(freqs_sin.T)
    # row p of a 64-row head block: pair j = 16*(p//32 % 2) + p%16, sign -/+ for e/o
    cos128 = np.empty((128, T), dtype=np.float32)
    sin128s = np.empty((128, T), dtype=np.float32)
    for p in range(128):
        ph = p % 64
        q32, i = divmod(ph, 32)
        j = 16 * q32 + (i % 16)
        cos128[p] = cosT[j]
        sin128s[p] = sinT[j] * (-1.0 if i < 16 else 1.0)
    # pre-shuffled sin: device computes u = raw*sin2, then swap(u) equals
    # raw[p^16]*sin_signed[p]
    swap16 = np.array([(p // 32) * 32 + ((p % 32) + 16) % 32 for p in range(128)])
    sin2 = sin128s[swap16]
    kk, qq = np.meshgrid(np.arange(128), np.arange(128), indexing="ij")
    tri = (kk <= qq).astype(np.float32)
    tri2 = np.concatenate([tri, tri], axis=1)  # [128, 256] for both heads

    w3 = w_qkv.reshape(D, 3, H, HD)
    in_maps = []
    for core in range(8):
        b, g = divmod(core, G)
        heads = np.arange(g * HPG, (g + 1) * HPG)
        wq = w3[:, 0, heads][:, :, perm].reshape(D, DG)
        wk = w3[:, 1, heads][:, :, perm].reshape(D, DG)
        wqk = np.ascontiguousarray(np.concatenate([wq, wk], axis=1))
        wv = np.ascontiguousarray(w3[:, 2, heads].reshape(D, DG))
        wo = np.ascontiguousarray(w_out.reshape(H, HD, D)[heads].reshape(DG, D))
        def sb_layout(a, kc=KC):
            # [128*kc, F] -> [128, kc, F] with partition-major contiguity
            return np.ascontiguousarray(
                a.reshape(kc, 128, -1).transpose(1, 0, 2)).astype(NPBF)
        in_maps.append({
            "xT": sb_layout(x[b].T),
            "wqk": sb_layout(wqk),
            "wv": sb_layout(wv),
            "wout": sb_layout(wo, kc=2),
            "cos128": cos128.astype(NPBF),
            "sin128s": sin2.astype(NPBF),
            "tri2": tri2.astype(NPBF),
        })
    return in_maps


def _run(in_maps, **kw):
    if "nc" not in _CACHE:
        _CACHE["nc"] = _build()
    return run_bass_kernel_spmd(_CACHE["nc"], in_maps, core_ids=list(range(8)), **kw)


def kernel(x, w_qkv, w_out, freqs_cos, freqs_sin):
    x = np.asarray(x, dtype=np.float32)
    w_qkv = np.asarray(w_qkv, dtype=np.float32)
    w_out = np.asarray(w_out, dtype=np.float32)
    freqs_cos = np.asarray(freqs_cos, dtype=np.float32)
    freqs_sin = np.asarray(freqs_sin, dtype=np.float32)

    in_maps = _prepare_shards(x, w_qkv, w_out, freqs_cos, freqs_sin)
    res = _run(in_maps)
    out = np.zeros((B, T, D), dtype=np.float64)
    for core in range(8):
        out[core // G] += res.results[core]["out"].astype(np.float64)
    return out.astype(np.float32)


# revision 26
# speedup vs baseline: 1.4990x; 1.0169x over previous
"""Causal self-attention (B=2,T=2048,D=1024,H=16,HD=64) + RoPE on 8 TRN2 NeuronCores.

Sharding: core = b*4 + g  (b: batch, g: head-group of 4 heads).
Each core computes QKV projection for its 4 heads, causal attention, and a
partial out-projection (rank-256 contribution). Host sums the 4 partials per
batch (the "all-reduce after out_proj").

v2: all matmul operands bf16 (same PE streaming rate as fp32r, but FWL weight
loads, half DMA bytes, 2x DVE modes). RoPE partner-swap moved off DVE onto
SBUF->SBUF DMA; causal masks on gpsimd; exp on ACT (the only exp engine).
Attention j-steps are software-pipelined (scores j+1 issued before PV j) and
interleaved with QKV/out-proj matmul "filler" so the PE never idles long
enough for HAM to re-throttle to half clock.
"""
import numpy as np
import ml_dtypes

import concourse.bass as bass
import concourse.mybir as mybir
from concourse import bacc
from concourse.tile import TileContext
from concourse.bass_utils import run_bass_kernel_spmd

B, T, D, H = 2, 2048, 1024, 16
HD = D // H            # 64
G = 4                  # head groups (tensor-parallel factor)
HPG = H // G           # 4 heads per group
DG = HPG * HD          # 256 head-dims per group
KC = D // 128          # 8 contraction chunks for D
NT = T // 512          # 4 T-chunks of 512
TT = T // 128          # 16 T-tiles of 128
F32 = mybir.dt.float32
BF = mybir.dt.bfloat16
NPBF = ml_dtypes.bfloat16

_CACHE = {}
DEBUG_DUMPS = False


def _build():
    nc = bacc.Bacc("TRN2", target_bir_lowering=False, debug=False, num_devices=8)

    xT_d = nc.dram_tensor("xT", [128, KC, T], BF, kind="ExternalInput").ap()
    wqk_d = nc.dram_tensor("wqk", [128, KC, 2 * DG], BF, kind="ExternalInput").ap()
    wv_d = nc.dram_tensor("wv", [128, KC, DG], BF, kind="ExternalInput").ap()
    wout_d = nc.dram_tensor("wout", [128, 2, D], BF, kind="ExternalInput").ap()
    cos_d = nc.dram_tensor("cos128", [128, T], BF, kind="ExternalInput").ap()
    sin_d = nc.dram_tensor("sin128s", [128, T], BF, kind="ExternalInput").ap()
    tri_d = nc.dram_tensor("tri2", [128, 256], BF, kind="ExternalInput").ap()
    out_d = nc.dram_tensor("out", [T, D], BF, kind="ExternalOutput").ap()

    with TileContext(nc) as tc:
        with (
            tc.tile_pool(name="const", bufs=1) as cpool,
            tc.tile_pool(name="big", bufs=1) as big,
            tc.tile_pool(name="rope", bufs=2) as ropep,
            tc.tile_pool(name="work", bufs=2) as work,
            tc.tile_pool(name="expp", bufs=3) as expp,
            tc.tile_pool(name="outp", bufs=2) as outp,
            tc.tile_pool(name="ps_mm", bufs=2, space="PSUM") as ps_mm,
            tc.tile_pool(name="ps_sc", bufs=2, space="PSUM") as ps_sc,
            tc.tile_pool(name="ps_pv0", bufs=1, space="PSUM") as ps_pv0,
            tc.tile_pool(name="ps_pv1", bufs=1, space="PSUM") as ps_pv1,
        ):
            cos_sb = cpool.tile([128, T], BF)
            sin_sb = cpool.tile([128, T], BF)
            tri_sb = cpool.tile([128, 256], BF)
            xT_sb = big.tile([128, KC, T], BF)
            wqk_sb = big.tile([128, KC, 2 * DG], BF)
            wv_sb = big.tile([128, KC, DG], BF)
            wout_sb = big.tile([128, 2, D], BF)
            # first q/k matmuls need wqk + xT chunk 0; order DMAs accordingly
            # only first-needed tensors upfront; xT chunks 1-3 and wout are
            # prefetched from inside earlier filler units so the HW DMA
            # queues don't round-robin everything to a late joint finish
            nc.sync.dma_start(out=wqk_sb[:], in_=wqk_d)
            nc.sync.dma_start(out=xT_sb[:, :, 0:512], in_=xT_d[:, :, 0:512])
            nc.sync.dma_start(out=wv_sb[:], in_=wv_d)
            nc.sync.dma_start(out=cos_sb[:], in_=cos_d)
            nc.sync.dma_start(out=sin_sb[:], in_=sin_d)
            nc.sync.dma_start(out=tri_sb[:], in_=tri_d)

            def stage_dma(n):
                if n < NT:
                    nc.sync.dma_start(
                        out=xT_sb[:, :, n * 512:(n + 1) * 512],
                        in_=xT_d[:, :, n * 512:(n + 1) * 512])
                if n == 2:
                    nc.sync.dma_start(out=wout_sb[:], in_=wout_d)

            # PE warm-up: dummy matmuls fill the DMA lead-in so HAM unthrottles
            warm_sb = cpool.tile([128, 256], BF)
            nc.vector.memset(warm_sb[:].bitcast(F32), 0.0)
            for w in range(40):
                wp = ps_mm.tile([128, 512], F32, tag="mm")
                nc.tensor.matmul(
                    wp[:, 0:256], lhsT=warm_sb[:, 0:128], rhs=warm_sb[:],
                    start=True, stop=True,
                )

            # qkT_sb m-index: 0,1 = q head-pairs (0,1),(2,3); 2,3 = k pairs
            qkT_sb = big.tile([128, 4, T], BF)
            # HD+2 so the ones columns pair up into one f32 lane for memset:
            # 0x3F803F80 == two bf16 1.0s
            v_sb = big.tile([128, TT, HPG, HD + 2], BF)
            ones_f32 = float(np.frombuffer(
                np.uint32(0x3F803F80).tobytes(), dtype=np.float32)[0])
            nc.vector.memset(
                v_sb[:, :, :, HD:HD + 2].bitcast(F32), ones_f32)
            outT_sb = big.tile([128, 2, T], BF)

            def qk_unit(n, m, t1, uu):
                # rot(x)[p] = x[p]*cos[p] + x[p^16]*sin_signed[p]; with
                # sin2[p] = sin_signed[p^16] pre-applied, swap(x*sin2) gives
                # the second term. Single PSUM reader (the copy) so the mm
                # bank releases fast for the next filler unit.
                ns = slice(n * 512, (n + 1) * 512)
                ps = ps_mm.tile([128, 512], F32, tag="mm")
                for k in range(KC):
                    nc.tensor.matmul(
                        ps[:],
                        lhsT=wqk_sb[:, k, m * 128:(m + 1) * 128],
                        rhs=xT_sb[:, k, ns],
                        start=(k == 0),
                        stop=(k == KC - 1),
                    )
                raw = ropep.tile([128, 512], BF, tag="raw")
                nc.vector.tensor_copy(raw[:], ps[:])
                nc.vector.tensor_mul(uu[:, m, :], raw[:], sin_sb[:, ns])
                nc.vector.tensor_mul(t1[:, m, :], raw[:], cos_sb[:, ns])

            def rope_finish(n, t1, uu):
                # partner swap p <-> p^16 via SBUF->SBUF DMA (frees DVE)
                ns = slice(n * 512, (n + 1) * 512)
                us = ropep.tile([128, 4, 512], BF, tag="us")
                for q in range(4):
                    nc.sync.dma_start(
                        out=us[32 * q + 16:32 * q + 32], in_=uu[32 * q:32 * q + 16])
                    nc.sync.dma_start(
                        out=us[32 * q:32 * q + 16], in_=uu[32 * q + 16:32 * q + 32])
                nc.vector.tensor_add(qkT_sb[:, :, ns], t1[:], us[:])

            def v_unit(n, j):
                ps = ps_mm.tile([128, 256], F32, tag="mm")
                for k in range(KC):
                    nc.tensor.matmul(
                        ps[:],
                        lhsT=xT_sb[:, k, j * 128:(j + 1) * 128],
                        rhs=wv_sb[:, k, :],
                        start=(k == 0),
                        stop=(k == KC - 1),
                    )
                nc.vector.tensor_copy(
                    v_sb[:, j, :, 0:HD], ps[:].rearrange("p (h d) -> p h d", h=HPG)
                )

            def qkv_units(n):
                """Filler units (closures) for projecting chunk n."""
                st = [None, None]

                def qk_closure(m):
                    def f():
                        if m == 0:
                            stage_dma(n + 1)
                            t1_t = ropep.tile([128, 4, 512], BF, tag="t1")
                            uu_t = ropep.tile([128, 4, 512], BF, tag="uu")
                            st[0], st[1] = t1_t, uu_t
                        qk_unit(n, m, st[0], st[1])
                        if m == 3:
                            rope_finish(n, st[0], st[1])
                    return f
                return [qk_closure(m) for m in range(4)] + \
                       [lambda j=j: v_unit(n, j) for j in range(4 * n, 4 * n + 4)]

            def proj_unit(t, nh):
                ps = ps_mm.tile([128, 512], F32, tag="mm")
                for c in range(2):
                    nc.tensor.matmul(
                        ps[:],
                        lhsT=outT_sb[:, c, t * 128:(t + 1) * 128],
                        rhs=wout_sb[:, c, nh * 512:(nh + 1) * 512],
                        start=(c == 0),
                        stop=(c == 1),
                    )
                ot = outp.tile([128, 512], BF, tag="ot")
                if (t + nh) % 2 == 0:
                    nc.scalar.copy(out=ot[:], in_=ps[:])
                else:
                    nc.vector.tensor_copy(ot[:], ps[:])
                nc.sync.dma_start(
                    out=out_d[t * 128:(t + 1) * 128, nh * 512:(nh + 1) * 512],
                    in_=ot[:],
                )

            def proj_units(g):
                return [lambda t=t, nh=nh: proj_unit(t, nh)
                        for t in range(4 * g, 4 * g + 4) for nh in range(2)]

            fill_q = []

            def pump(k=1):
                for _ in range(k):
                    if fill_q:
                        fill_q.pop(0)()

            def attn_group(g, pat, last=False):
                """pat(step) -> how many filler units to pump after that step."""
                step = [0]
                for hp in range(2):
                    qm, km = hp, 2 + hp
                    pv0 = ps_pv0.tile([65, 512], F32, tag="pv0")
                    pv1 = ps_pv1.tile([65, 512], F32, tag="pv1")
                    jmax = 4 * g + 3
                    # diag blocks first so the causal masks (and their engine
                    # hops) sit early in the stream, not on the tail chain
                    j_order = list(range(4 * g, 4 * g + 4)) + list(range(4 * g))
                    exs = {}

                    def pv_step(idx):
                        j = j_order[idx]
                        ex = exs.pop(j)
                        d = j - 4 * g
                        nstart = 128 * d if d > 0 else 0
                        ncols = 512 - nstart
                        for half, pv in ((0, pv0), (1, pv1)):
                            nc.tensor.matmul(
                                pv[:, nstart:512],
                                lhsT=v_sb[:, j, 2 * hp + half, 0:HD + 1],
                                rhs=ex[:, half * 512:half * 512 + ncols],
                                start=(idx == 0),
                                stop=(idx == jmax),
                            )

                    for idx in range(jmax + 1):
                        j = j_order[idx]
                        d = j - 4 * g
                        nstart = 128 * d if d > 0 else 0
                        ncols = 512 - nstart
                        ex = expp.tile([128, 1024], BF, tag="ex")
                        exs[j] = ex
                        # two heads' score matmuls packed into one PE pass
                        # (row groups 0-1 / 2-3 run concurrently)
                        sc = ps_sc.tile([128, 1024], F32, tag="sc")
                        for half in range(2):
                            pb = 64 * half
                            nc.tensor.matmul(
                                sc[:, half * 512:half * 512 + ncols],
                                lhsT=qkT_sb[pb:pb + 64, km, j * 128:(j + 1) * 128],
                                rhs=qkT_sb[pb:pb + 64, qm,
                                           g * 512 + nstart:(g + 1) * 512],
                                start=True,
                                stop=True,
                            )
                        if ncols == 512:
                            nc.scalar.activation(
                                ex[:], sc[:],
                                mybir.ActivationFunctionType.Exp, scale=0.125,
                            )
                        else:
                            exv = ex[:].rearrange("p (u c) -> p u c", u=2)[:, :, 0:ncols]
                            scv = sc[:].rearrange("p (u c) -> p u c", u=2)[:, :, 0:ncols]
                            nc.scalar.activation(
                                exv, scv, mybir.ActivationFunctionType.Exp, scale=0.125,
                            )
                        if d >= 0:
                            # causal mask on the diagonal 128x128 of both heads
                            exm = ex[:].rearrange("p (u c) -> p u c", u=2)[:, :, 0:128]
                            trm = tri_sb[:].rearrange("p (u c) -> p u c", u=2)
                            nc.vector.tensor_mul(exm, exm, trm)
                        if idx > 0:
                            pv_step(idx - 1)
                        pump(pat(step[0]))
                        step[0] += 1
                    pv_step(jmax)
                    if not (last and hp == 1):
                        pump(2)

                    # interleave the halves' chains: both gpsimd broadcasts
                    # issue early and overlap the vector-side work
                    recs, recbs = [], []
                    for half, pv in ((0, pv0), (1, pv1)):
                        # reciprocal_approx_fast misreads PSUM operands; den
                        # must be staged through SBUF first (scalar queue --
                        # it has idle windows between exps)
                        den = work.tile([1, 512], F32, tag="den")
                        nc.scalar.copy(out=den[:], in_=pv[64:65, :])
                        rec = work.tile([1, 512], F32, tag="rec")
                        nc.vector.reciprocal_approx_fast(rec[:], den[:])
                        recs.append(rec)
                    for half in range(2):
                        recb = work.tile([64, 512], F32, tag="recb")
                        nc.gpsimd.partition_broadcast(
                            recb[:], recs[half][0:1, :], channels=64)
                        recbs.append(recb)
                    for half, pv in ((0, pv0), (1, pv1)):
                        pb = 64 * half
                        nc.vector.tensor_mul(
                            outT_sb[pb:pb + 64, hp, g * 512:(g + 1) * 512],
                            pv[0:64, :],
                            recbs[half][:],
                        )

            # lead-in: project chunk 0 densely, then attention with filler
            for u in qkv_units(0):
                u()
            fill_q.extend(qkv_units(1))
            attn_group(0, lambda s: 1)            # 8 steps, 8 units
            fill_q.extend(qkv_units(2))
            attn_group(1, lambda s: s % 2)        # 16 steps, 8 units
            fill_q.extend(qkv_units(3))
            fill_q.extend(proj_units(0))
            attn_group(2, lambda s: 1 if s < 16 else 0)  # 24 steps, 16 units
            fill_q.extend(proj_units(1))
            fill_q.extend(proj_units(2))
            attn_group(3, lambda s: (s + 1) % 2, last=True)  # 32 steps, 16 units
            pump(len(fill_q))
            for u in proj_units(3):
                u()

            if DEBUG_DUMPS:
                dq = nc.dram_tensor("dbg_qk", [128, 4, T], BF, kind="ExternalOutput").ap()
                dv = nc.dram_tensor("dbg_v", [128, TT, HPG, HD + 2], BF, kind="ExternalOutput").ap()
                do = nc.dram_tensor("dbg_outT", [128, 2, T], BF, kind="ExternalOutput").ap()
                nc.sync.dma_start(out=dq[:], in_=qkT_sb[:])
                nc.sync.dma_start(out=dv[:], in_=v_sb[:])
                nc.sync.dma_start(out=do[:], in_=outT_sb[:])

    nc.compile()
    return nc


def _qk_perm():
    """hd permutation for q/k columns: RoPE pair j -> (e,o) rows 16-interleaved
    so the swap stays within 32-partition quadrants (p <-> p^16)."""
    perm = np.empty(HD, dtype=np.int64)
    for p in range(HD):
        q32, i = divmod(p, 32)
        j = 16 * q32 + (i % 16)
        perm[p] = 2 * j + (1 if i >= 16 else 0)
    return perm


def _prepare_shards(x, w_qkv, w_out, freqs_cos, freqs_sin):
    perm = _qk_perm()
    cosT = np.ascontiguousarray(freqs_cos.T)  # [32, T]
    sinT = np.ascontiguousarray(freqs_sin.T)
    # row p of a 64-row head block: pair j = 16*(p//32 % 2) + p%16, sign -/+ for e/o
    cos128 = np.empty((128, T), dtype=np.float32)
    sin128s = np.empty((128, T), dtype=np.float32)
    for p in range(128):
        ph = p % 64
        q32, i = divmod(ph, 32)
        j = 16 * q32 + (i % 16)
        cos128[p] = cosT[j]
        sin128s[p] = sinT[j] * (-1.0 if i < 16 else 1.0)
    # pre-shuffled sin: device computes u = raw*sin2, then swap(u) equals
    # raw[p^16]*sin_signed[p]
    swap16 = np.array([(p // 32) * 32 + ((p % 32) + 16) % 32 for p in range(128)])
    sin2 = sin128s[swap16]
    kk, qq = np.meshgrid(np.arange(128), np.arange(128), indexing="ij")
    tri = (kk <= qq).astype(np.float32)
    tri2 = np.concatenate([tri, tri], axis=1)  # [128, 256] for both heads

    w3 = w_qkv.reshape(D, 3, H, HD)
    in_maps = []
    for core in range(8):
        b, g = divmod(core, G)
        heads = np.arange(g * HPG, (g + 1) * HPG)
        wq = w3[:, 0, heads][:, :, perm].reshape(D, DG)
        wk = w3[:, 1, heads][:, :, perm].reshape(D, DG)
        wqk = np.ascontiguousarray(np.concatenate([wq, wk], axis=1))
        wv = np.ascontiguousarray(w3[:, 2, heads].reshape(D, DG))
        wo = np.ascontiguousarray(w_out.reshape(H, HD, D)[heads].reshape(DG, D))
        def sb_layout(a, kc=KC):
            # [128*kc, F] -> [128, kc, F] with partition-major contiguity
            return np.ascontiguousarray(
                a.reshape(kc, 128, -1).transpose(1, 0, 2)).astype(NPBF)
        in_maps.append({
            "xT": sb_layout(x[b].T),
            "wqk": sb_layout(wqk),
            "wv": sb_layout(wv),
            "wout": sb_layout(wo, kc=2),
            "cos128": cos128.astype(NPBF),
            "sin128s": sin2.astype(NPBF),
            "tri2": tri2.astype(NPBF),
        })
    return in_maps


def _run(in_maps, **kw):
    if "nc" not in _CACHE:
        _CACHE["nc"] = _build()
    return run_bass_kernel_spmd(_CACHE["nc"], in_maps, core_ids=list(range(8)), **kw)


def kernel(x, w_qkv, w_out, freqs_cos, freqs_sin):
    x = np.asarray(x, dtype=np.float32)
    w_qkv = np.asarray(w_qkv, dtype=np.float32)
    w_out = np.asarray(w_out, dtype=np.float32)
    freqs_cos = np.asarray(freqs_cos, dtype=np.float32)
    freqs_sin = np.asarray(freqs_sin, dtype=np.float32)

    in_maps = _prepare_shards(x, w_qkv, w_out, freqs_cos, freqs_sin)
    res = _run(in_maps)
    out = np.zeros((B, T, D), dtype=np.float64)
    for core in range(8):
        out[core // G] += res.results[core]["out"].astype(np.float64)
    return out.astype(np.float32)
